# revision 16
# baseline (speedup 1.0000x reference)
"""Trainium2 Bass kernel for nn_ButterflyLayer2D (butterfly 2D CNN).

Strategy: pure data parallel over 8 NeuronCores (16 batch each).

Layout invariant: activations live in SBUF as [128 = (w%2)*64 + c,
(node, b, h, w//2)] bf16.  Each per-node 2x2-stride-2 conv is then a set
of matmuls: contraction K=128=(dw,c), accumulated over x=dh (start/stop),
with output w-parity q selected by slicing the rhs w2 dim (q::2) and
writing psum rows q*64+c via col tile_position (0, q*64).  The psum rows
(q, c_out) are exactly the next level's partition layout, so every
eviction is a contiguous full-width relu+bias op.

PSUM tiles are [128, 2048] (4 banks, ring of 2).  Within a tile the
matmuls are ordered x-outer so one weight image (w[n,x] in both column
halves) serves 4 consecutive 512-col streams, hiding the ~130ns
no-FWL LDWEIGHTS behind the previous x-phase.  Evictions are split as
2x [128, 1024] ops, one to ScalarE and one to VectorE, so both engines
drain a tile in parallel (psum reads interleave across 2 banks).

When a level's bias is nonzero, multi-node tiles fall back to per-node
evictions (bias must be per-partition-constant within one evict op).
"""

import numpy as np
from contextlib import ExitStack

import concourse.bass as bass
import concourse.tile as tile
from concourse import bacc, mybir
from concourse.bass_utils import run_bass_kernel_spmd

F32 = mybir.dt.float32
BF16 = mybir.dt.bfloat16
AF = mybir.ActivationFunctionType
ALU = mybir.AluOpType

B, IN, NLVL, KLVL, C = 128, 256, 6, 3, 64
NK, OU, OV = 8, 8, 8
NCORES = 8
BC = B // NCORES          # 16 per-core batch
BG = BC
HALF = BG // 4
LVL_NODES = [4, 16, 64, 64, 64, 64]
LVL_HIN = [64, 32, 16, 8, 4, 2]
WCH = 16                  # weight streaming chunk (nodes)
BIAS_OFF = {0: 0, 1: 1, 2: 5, 3: 21, 4: 85, 5: 149, 6: 213}  # cols in bias blob


# ----------------------------------------------------------------------------
# host-side pre-arrangement
# ----------------------------------------------------------------------------

def _prep_weights(inputs):
    """Weights/biases blobs shared by all cores."""
    import ml_dtypes

    out = {}
    zflags = {}
    fin = inputs["in_filter"][:, :, 0, :].reshape(16, C).astype(np.float32)
    finr = np.zeros((128, C), np.float32)
    for g in range(4):
        finr[g * 32 : g * 32 + 16] = fin
    out["fin"] = finr.astype(ml_dtypes.bfloat16)

    bias_cols = [np.concatenate([inputs["in_bias"], inputs["in_bias"]])
                 .reshape(128, 1).astype(np.float32)]
    for lvl in range(1, NLVL + 1):
        f = inputs[f"f{lvl}"].astype(np.float32)  # [n,n,2,2,C,C] (x=dh,y=dw,ci,co)
        n = f.shape[0]
        assert n == 2 ** min(lvl, KLVL)
        w = f.transpose(0, 1, 3, 4, 2, 5).reshape(n * n, 2 * C, 2 * C)
        out[f"w{lvl}"] = np.ascontiguousarray(w.transpose(1, 0, 2)).reshape(
            128, n * n * 128
        ).astype(ml_dtypes.bfloat16)
        b = inputs[f"b{lvl}"].astype(np.float32).reshape(n * n, C)
        zflags[lvl] = not np.any(b)
        if lvl < NLVL:
            bb = np.concatenate([b, b], axis=1)  # rows (q,c), dup across q
            bias_cols.append(np.ascontiguousarray(bb.T))
        else:
            bb = b.reshape(n * n // 2, 2 * C)    # rows (cEven,cOdd) per pair
            bias_cols.append(np.ascontiguousarray(bb.T))
    out["bb"] = np.concatenate(bias_cols, axis=1)  # [128, 245]

    # dense, stacked pairs: rows 0:64 = c for even node, 64:128 = odd;
    # cols (pair, (r,ou,ov))
    wd = inputs["Wd"].astype(np.float32).reshape(NK * NK, 2, C, OU * OV)
    wdn = wd.transpose(0, 2, 1, 3).reshape(NK * NK, C, 2 * OU * OV)
    wds = np.zeros((128, (NK * NK // 2) * 2 * OU * OV), np.float32)
    for p in range(NK * NK // 2):
        wds[0:64, p * 128 : (p + 1) * 128] = wdn[2 * p]
        wds[64:128, p * 128 : (p + 1) * 128] = wdn[2 * p + 1]
    out["wd"] = np.ascontiguousarray(wds).astype(ml_dtypes.bfloat16)
    return out, zflags


def _prep_input(in_data_core):
    """Per-core input blob: [64 = (b%4)*16 + (i%4)*4 + (j%4),
    (b//4, x=i//4, y4=j//4)] packed (no zero rows)."""
    import ml_dtypes

    ind = in_data_core[:, :, :, 0]  # [16, 256, 256]
    a = ind.reshape(HALF, 4, 64, 4, 64, 4)      # [half, g, x, p, y4, q]
    a = a.transpose(1, 3, 5, 0, 2, 4)           # [g, p, q, half, x, y4]
    return np.ascontiguousarray(a).reshape(64, HALF * 64 * 64).astype(
        ml_dtypes.bfloat16
    )


def _decode_output(t2_core):
    """t2 [128=(r,ou,ov), (par, pair, bl)], node=2*pair+par -> [16, 64, 64, 2]."""
    t = t2_core.reshape(2, OU, OV, 2, 32, BG)       # r,ou,ov,par,pair,bl
    t = t.transpose(0, 1, 2, 4, 3, 5)               # r,ou,ov,pair,par,bl
    t = t.reshape(2, OU, OV, NK, NK, BG)            # r,ou,ov,u,v,bl
    t = t.transpose(5, 3, 1, 4, 2, 0)               # bl,u,ou,v,ov,r
    return np.ascontiguousarray(t).reshape(BG, NK * OU, NK * OV, 2)


# ----------------------------------------------------------------------------
# device kernel
# ----------------------------------------------------------------------------

def _build_kernel(zflags, debug=False):
    nc = bacc.Bacc(None, target_bir_lowering=False)
    p = {}
    p["a0"] = nc.declare_dram_parameter("a0", [64, HALF * 64 * 64], BF16, isOutput=False)
    p["fin"] = nc.declare_dram_parameter("fin", [128, C], BF16, isOutput=False)
    p["bb"] = nc.declare_dram_parameter("bb", [128, 245], F32, isOutput=False)
    for lvl in range(1, NLVL + 1):
        n2 = LVL_NODES[lvl - 1]
        p[f"w{lvl}"] = nc.declare_dram_parameter(f"w{lvl}", [128, n2 * 128], BF16, isOutput=False)
    p["wd"] = nc.declare_dram_parameter("wd", [128, 32 * 128], BF16, isOutput=False)
    t2 = nc.declare_dram_parameter("t2", [128, NK * NK * BG], F32, isOutput=True)
    dbg = {}
    if debug:
        dbg["X"] = nc.declare_dram_parameter("dbgX", [128, BG * 64 * 32], BF16, isOutput=True)
        for lvl in range(1, 6):
            n2 = LVL_NODES[lvl - 1]
            Ho = LVL_HIN[lvl - 1] // 2
            dbg[lvl] = nc.declare_dram_parameter(
                f"dbgL{lvl}", [128, n2 * BG * Ho * max(Ho // 2, 1)], BF16, isOutput=True)
        dbg["F"] = nc.declare_dram_parameter("dbgF", [128, 32 * BG], BF16, isOutput=True)

    evict_ctr = [0]

    with tile.TileContext(nc) as tc, ExitStack() as ctx:
        const = ctx.enter_context(tc.tile_pool(name="const", bufs=1))
        wpool = ctx.enter_context(tc.tile_pool(name="wts", bufs=8))
        apool = ctx.enter_context(tc.tile_pool(name="acts", bufs=1))
        inpool = ctx.enter_context(tc.tile_pool(name="inp", bufs=1))
        fpool = ctx.enter_context(tc.tile_pool(name="feat", bufs=1))
        ppool = ctx.enter_context(tc.tile_pool(name="ps", bufs=2, space="PSUM"))

        # --- startup DMAs: fin, input chunks, bias blob, then weights ---
        fin_t = const.tile([128, C], BF16)
        nc.sync.dma_start(fin_t[:], p["fin"][:])
        a0s = inpool.tile([128, HALF * 64 * 64], BF16, tag="a0s", name="a0s")
        for g in range(4):
            nc.sync.dma_start(
                a0s[g * 32 : g * 32 + 16, :], p["a0"][g * 16 : (g + 1) * 16, :]
            )
        a0v = a0s[:].rearrange("p (h x y) -> p h x y", h=HALF, x=64)
        bb_t = const.tile([128, 245], F32, tag="bb", name="bb")
        nc.sync.dma_start(bb_t[:], p["bb"][:])

        def bias_ap(lvl, n):
            off = BIAS_OFF[lvl] + n
            return bb_t[:, off : off + 1]

        def evict(out_ap, psum_ap, b_ap):
            """relu(psum + bias) -> sbuf, alternating engines."""
            evict_ctr[0] += 1
            if evict_ctr[0] % 2 == 0:
                if b_ap is None:
                    nc.scalar.activation(out_ap, psum_ap, AF.Relu)
                else:
                    nc.scalar.activation(out_ap, psum_ap, AF.Relu, bias=b_ap)
            else:
                if b_ap is None:
                    nc.vector.tensor_scalar(out_ap, psum_ap, 0.0, None, op0=ALU.max)
                else:
                    nc.vector.tensor_scalar(out_ap, psum_ap, b_ap, 0.0,
                                            op0=ALU.add, op1=ALU.max)

        def wchunk(lvl, g0, gn):
            wlt = wpool.tile([128, WCH * 128], BF16, tag="wch", name=f"w{lvl}_{g0}")
            src = p["wd"] if lvl == "d" else p[f"w{lvl}"]
            nc.sync.dma_start(
                wlt[:, : gn * 128], src[:, g0 * 128 : (g0 + gn) * 128]
            )
            return wlt

        # ---------------- input conv + L1, interleaved ----------------
        X = apool.tile([128, BG * 64 * 32], BF16, tag="sA", name="x0")
        Xv = X[:].rearrange("p (b h w) -> p b h w", b=BG, h=64)
        L1n = LVL_NODES[0]
        L1out = apool.tile([128, L1n * BG * 32 * 16], BF16, tag="sB", name="a1")
        L1v = L1out[:].rearrange("p (n b h w) -> p n b h w", n=L1n, b=BG, h=32)
        w1t = wchunk(1, 0, L1n)

        def input_bl(bl):
            g, half = bl % 4, bl // 4
            pt = ppool.tile([128, 2048], F32, tag="ps", name=f"pin{bl}")
            for xq in range(4):
                for q in (0, 1):
                    rhs = a0v[g * 32 : g * 32 + 16, half,
                              xq * 16 : (xq + 1) * 16, q::2]
                    nc.tensor.matmul(
                        pt[q * 64 : (q + 1) * 64, xq * 512 : (xq + 1) * 512],
                        fin_t[g * 32 : g * 32 + 16, :],
                        rhs,
                        start=True, stop=True,
                        tile_position=(g * 32, q * 64),
                    )
            for h2 in (0, 1):
                evict(Xv[:, bl, h2 * 32 : (h2 + 1) * 32, :],
                      pt[:, h2 * 1024 : (h2 + 1) * 1024], bias_ap(0, 0))

        def l1_tile(n, t):
            # one node, bl-chunk 4t..4t+4, x-outer
            pt = ppool.tile([128, 2048], F32, tag="ps", name=f"p1_{n}_{t}")
            for x in (0, 1):
                for j in range(4):
                    bl = 4 * t + j
                    for q in (0, 1):
                        nc.tensor.matmul(
                            pt[q * 64 : (q + 1) * 64, j * 512 : (j + 1) * 512],
                            w1t[:, n * 128 + x * 64 : n * 128 + (x + 1) * 64],
                            Xv[:, bl, x::2, q::2],
                            start=(x == 0), stop=(x == 1),
                            skip_group_check=True,
                            tile_position=(0, q * 64),
                        )
            for h2 in (0, 1):
                evict(L1v[:, n, 4 * t + 2 * h2 : 4 * t + 2 * h2 + 2, :, :],
                      pt[:, h2 * 1024 : (h2 + 1) * 1024], bias_ap(1, n))

        for t in range(4):
            for bl in range(4 * t, 4 * t + 4):
                input_bl(bl)
            if t >= 1:
                for n in range(L1n):
                    l1_tile(n, t - 1)
        for n in range(L1n):
            l1_tile(n, 3)
        if debug:
            nc.sync.dma_start(dbg["X"][:], X[:])
            nc.sync.dma_start(dbg[1][:], L1out[:])

        # ---------------- levels 2..5 ----------------
        cur, cur_nodes = L1out, L1n
        tags = {2: "sA", 3: "sB", 4: "sA", 5: "sB"}
        for lvl in range(2, 6):
            n2 = LVL_NODES[lvl - 1]
            grid = int(np.sqrt(n2))
            Hin = LVL_HIN[lvl - 1]
            Ho, W2o = Hin // 2, Hin // 4
            pcols = BG * Ho * W2o               # output cols per node
            npt = max(1, 2048 // pcols)         # nodes per psum tile
            zb = zflags[lvl]
            nxt = apool.tile([128, n2 * BG * Ho * W2o], BF16,
                             tag=tags[lvl], name=f"a{lvl}")
            curv = cur[:].rearrange("p (n b h w) -> p n b h w",
                                    n=cur_nodes, b=BG, h=Hin)
            nxtv = nxt[:].rearrange("p (n b h w) -> p n b h w",
                                    n=n2, b=BG, h=Ho)
            pgrid = int(np.sqrt(cur_nodes))

            def parent(n):
                if lvl <= KLVL:
                    return (n // grid // 2) * pgrid + (n % grid) // 2
                return n

            if pcols >= 2048:
                # one or more tiles per node; blocks are b-chunks of 512 cols
                tpn = pcols // 2048             # tiles per node
                bpt = BG // tpn                 # b per tile
                bpb = max(1, bpt // 4)          # b per 512-block
                for g0 in range(0, n2, WCH):
                    wlt = wchunk(lvl, g0, min(WCH, n2 - g0))
                    for n in range(g0, g0 + min(WCH, n2 - g0)):
                        ln = n - g0
                        pn = parent(n)
                        for tt in range(tpn):
                            pt = ppool.tile([128, 2048], F32, tag="ps",
                                            name=f"p{lvl}_{n}_{tt}")
                            for x in (0, 1):
                                for j in range(4):
                                    b0 = tt * bpt + j * bpb
                                    for q in (0, 1):
                                        nc.tensor.matmul(
                                            pt[q * 64 : (q + 1) * 64,
                                               j * 512 : (j + 1) * 512],
                                            wlt[:, ln * 128 + x * 64 :
                                                ln * 128 + (x + 1) * 64],
                                            curv[:, pn, b0 : b0 + bpb, x::2, q::2],
                                            start=(x == 0), stop=(x == 1),
                                            skip_group_check=True,
                                            tile_position=(0, q * 64),
                                        )
                            for h2 in (0, 1):
                                b0 = tt * bpt + h2 * (bpt // 2)
                                evict(nxtv[:, n, b0 : b0 + bpt // 2, :, :],
                                      pt[:, h2 * 1024 : (h2 + 1) * 1024],
                                      bias_ap(lvl, n))
            else:
                # multiple nodes per tile (npt = 4, 16 or 64)
                for m0 in range(0, n2, npt):
                    pt = ppool.tile([128, 2048], F32, tag="ps",
                                    name=f"p{lvl}_{m0}")
                    for n in range(m0, m0 + npt):
                        if n % WCH == 0:
                            wlt = wchunk(lvl, n, min(WCH, n2 - n))
                        ln = n % WCH
                        lt = n - m0
                        pn = parent(n)
                        for x in (0, 1):
                            for q in (0, 1):
                                nc.tensor.matmul(
                                    pt[q * 64 : (q + 1) * 64,
                                       lt * pcols : (lt + 1) * pcols],
                                    wlt[:, ln * 128 + x * 64 :
                                        ln * 128 + (x + 1) * 64],
                                    curv[:, pn, :, x::2, q::2],
                                    start=(x == 0), stop=(x == 1),
                                    skip_group_check=True,
                                    tile_position=(0, q * 64),
                                )
                    if zb:
                        for h2 in (0, 1):
                            evict(nxtv[:, m0 + h2 * (npt // 2) :
                                       m0 + (h2 + 1) * (npt // 2), :, :, :],
                                  pt[:, h2 * 1024 : (h2 + 1) * 1024], None)
                    else:
                        for n in range(m0, m0 + npt):
                            lt = n - m0
                            evict(nxtv[:, n, :, :, :],
                                  pt[:, lt * pcols : (lt + 1) * pcols],
                                  bias_ap(lvl, n))
            if debug:
                nc.sync.dma_start(dbg[lvl][:], nxt[:])
            cur, cur_nodes = nxt, n2

        # ---------------- level 6 (node pairs, M=64) ----------------
        F = fpool.tile([128, 32 * BG], BF16, tag="feats", name="feats")
        Fv = F[:].rearrange("p (pr b) -> p pr b", pr=32)
        curv = cur[:].rearrange("p (n b h w) -> p n b h w", n=64, b=BG, h=2)
        pt6 = ppool.tile([128, 512], F32, tag="ps", name="p6")
        for g0 in range(0, 64, WCH):
            w6t = wchunk(6, g0, WCH)
            for pr in range(g0 // 2, (g0 + WCH) // 2):
                for half in (0, 1):
                    node = 2 * pr + half
                    ln = node - g0
                    for x in (0, 1):
                        nc.tensor.matmul(
                            pt6[half * 64 : (half + 1) * 64,
                                pr * BG : (pr + 1) * BG],
                            w6t[:, ln * 128 + x * 64 : ln * 128 + (x + 1) * 64],
                            curv[:, node, :, x, 0],
                            start=(x == 0), stop=(x == 1),
                            skip_group_check=True,
                            tile_position=(0, half * 64),
                        )
        if zflags[6]:
            evict(F[:], pt6[:], None)
        else:
            for pr in range(32):
                evict(Fv[:, pr, :], pt6[:, pr * BG : (pr + 1) * BG],
                      bias_ap(6, pr))
        if debug:
            nc.sync.dma_start(dbg["F"][:], F[:])

        # ---------------- dense ----------------
        # t2 cols parity-major: (par, pair, b); node = 2*pair + par.
        t2s = fpool.tile([128, NK * NK * BG], F32, tag="t2s", name="t2s")
        for t in range(2):
            wdt = wchunk("d", t * 16, 16)
            for par in (0, 1):
                ptd = ppool.tile([128, 16 * BG], F32, tag="ps",
                                 name=f"pd{t}_{par}")
                for lp in range(16):
                    p_ = t * 16 + lp
                    nc.tensor.matmul(
                        ptd[:, lp * BG : (lp + 1) * BG],
                        wdt[par * 64 : (par + 1) * 64,
                            lp * 128 : (lp + 1) * 128],
                        Fv[par * 64 : (par + 1) * 64, p_, :],
                        start=True, stop=True,
                        tile_position=(par * 64, 0),
                    )
                evict_ctr[0] += 1
                dst = t2s[:, par * 512 + t * 256 : par * 512 + (t + 1) * 256]
                if evict_ctr[0] % 2 == 0:
                    nc.scalar.copy(dst, ptd[:])
                else:
                    nc.vector.tensor_copy(dst, ptd[:])
            nc.sync.dma_start(
                t2[:, t * 256 : t * 256 + 256], t2s[:, t * 256 : t * 256 + 256]
            )
            nc.sync.dma_start(
                t2[:, 512 + t * 256 : 512 + (t + 1) * 256],
                t2s[:, 512 + t * 256 : 512 + (t + 1) * 256],
            )
    nc.compile()
    return nc


# ----------------------------------------------------------------------------
# entry point
# ----------------------------------------------------------------------------

def kernel(**inputs):
    inputs = {k: np.asarray(v) for k, v in inputs.items()}
    wblobs, zflags = _prep_weights(inputs)
    nc = _build_kernel(zflags)
    in_maps = []
    for c in range(NCORES):
        m = dict(wblobs)
        m["a0"] = _prep_input(inputs["in_data"][c * BC : (c + 1) * BC])
        in_maps.append(m)
    res = run_bass_kernel_spmd(nc, in_maps, list(range(NCORES)))
    outs = [_decode_output(res.results[c]["t2"]) for c in range(NCORES)]
    return np.concatenate(outs, axis=0).astype(np.float32)


if __name__ == "__main__":
    import reference as ref

    inputs = {k: np.asarray(v) for k, v in ref.setup_inputs().items()}
    expected = np.asarray(ref.reference(**inputs))
    actual = kernel(**inputs)
    err = np.abs(actual - expected).max()
    rel = err / np.abs(expected).max()
    print("absmax:", err, "rel:", rel)
